# revision 95
# baseline (speedup 1.0000x reference)
import sys

sys.path.insert(0, "/opt/trn_rl_repo")

import math

import numpy as np
import ml_dtypes

import concourse.bass as bass
import concourse.mybir as mybir
import concourse.tile as tile
from concourse import bacc
from concourse.bass_utils import run_bass_kernel_spmd
from concourse.masks import make_identity

F32 = mybir.dt.float32
F32R = mybir.dt.float32r
BF16 = mybir.dt.bfloat16
FP8 = mybir.dt.float8e4
DR = mybir.MatmulPerfMode.DoubleRow
EXPF = mybir.ActivationFunctionType.Exp
RECIPF = mybir.ActivationFunctionType.Reciprocal

B, S, D = 8, 1024, 1024
N_H = 16
REL_K = 16
d_k = D // N_H  # 64
N_CORES = 8
MASKVAL = -1e30
BF = np.dtype(ml_dtypes.bfloat16)
E4 = np.dtype(ml_dtypes.float8_e4m3)

_CACHE = {}
TRACE = False


def build_module():
    nc = bacc.Bacc("TRN2", detect_race_conditions=False, num_swdge_queues=4)

    X8 = nc.dram_tensor("X8", [128, 16384], FP8, kind="ExternalInput")
    WQK1 = nc.dram_tensor("WQK1", [D, 4096], FP8, kind="ExternalInput")
    WQK2 = nc.dram_tensor("WQK2", [512, 4096], FP8, kind="ExternalInput")
    WV1 = nc.dram_tensor("WV1", [D, 2048], FP8, kind="ExternalInput")
    WV2 = nc.dram_tensor("WV2", [512, 2048], FP8, kind="ExternalInput")
    Wp = nc.dram_tensor("Wp", [D, D], BF16, kind="ExternalInput")
    bqk = nc.dram_tensor("bqk", [128, 16], F32, kind="ExternalInput")
    bprow = nc.dram_tensor("bprow", [1, D], BF16, kind="ExternalInput")
    dlut = nc.dram_tensor("dlut", [d_k, 16], BF16, kind="ExternalInput")
    dlv = nc.dram_tensor("dlv", [16, d_k], BF16, kind="ExternalInput")
    zbT = nc.dram_tensor("zbT", [128, 2561], BF16, kind="ExternalInput")
    OUT = nc.dram_tensor("OUT", [S, D], BF16, kind="ExternalOutput")

    zbP = [nc.dram_tensor(f"zbP{k}", [128, 2561], BF16) for k in range(2)]
    dtD = [nc.dram_tensor(f"dtD{k}", [128, 160], BF16) for k in range(4)]
    ewP = [nc.dram_tensor(f"ewP{k}", [128, 1153], BF16) for k in range(4)]

    with tile.TileContext(nc) as tc:
        with (
            tc.tile_pool(name="pers", bufs=1) as pers,
            tc.tile_pool(name="mm", bufs=4, space="PSUM") as mmp,
        ):
            # ---- resident loads first, split across queues ----
            # x8 holds fp8 [x1_d | t] cols 0..8191 and [x2_d | t] cols 8192..16383
            x8_sb = pers.tile([128, 16384], FP8, tag="x8")
            for c in range(4):
                eng = nc.sync if c % 2 == 0 else nc.scalar
                eng.dma_start(out=x8_sb[:, 4096 * c:4096 * (c + 1)],
                              in_=X8[:, 4096 * c:4096 * (c + 1)])

            def x_stat(d_off, pair_stride, col_off):
                # stationary [128, 2, 128] slice of x8 (vproj)
                return bass.AP(tensor=x8_sb[:].tensor,
                               offset=x8_sb[:].offset + d_off + col_off,
                               ap=[[16384, 128], [pair_stride, 2], [1, 128]])

            def x_mov(d_off, pair_stride, col_off):
                # moving [128, 2, 256] slice of x8 (qkproj)
                return bass.AP(tensor=x8_sb[:].tensor,
                               offset=x8_sb[:].offset + d_off + col_off,
                               ap=[[16384, 128], [pair_stride, 2], [1, 256]])

            # ---- constants ----
            identf = pers.tile([128, 128], F32)
            make_identity(nc, identf[:])
            identb = pers.tile([128, 128], BF16)
            nc.vector.tensor_copy(identb[:], identf[:])
            dlut_sb = pers.tile([128, 16], BF16)
            nc.scalar.dma_start(out=dlut_sb[0:64, :], in_=dlut[:])
            nc.scalar.dma_start(out=dlut_sb[64:128, :], in_=dlut[:])
            dlv_sb = pers.tile([16, d_k], BF16)
            nc.scalar.dma_start(out=dlv_sb[:], in_=dlv[:])
            bqk_sb = pers.tile([128, 16], F32)
            nc.sync.dma_start(out=bqk_sb[:], in_=bqk[:])
            bp_sb = pers.tile([1, D], BF16)
            nc.sync.dma_start(out=bp_sb[:], in_=bprow[:])
            ones_row = pers.tile([1, 512], BF16)
            nc.vector.memset(ones_row[:], 1.0)
            bpb_sb = pers.tile([128, 1024], BF16, tag="bpb")
            ones_col = pers.tile([1, 128], BF16)
            nc.vector.memset(ones_col[:], 1.0)
            ones_rf = pers.tile([1, 64], F32)
            nc.vector.memset(ones_rf[:], 1.0)
            ones_r = pers.tile([1, 64], F32R)
            nc.vector.tensor_copy(ones_r[:], ones_rf[:])

            # ---- v projection -> vhat_sb (65-stride layout + ones cols) ----
            vhat_sb = [pers.tile([128, 16 * 65], BF16, name=f"vh{jt}", tag=f"vh{jt}")
                       for jt in range(8)]
            with tc.tile_pool(name="wv", bufs=1) as wvp:
                # zbP init (template with causal mask / zeros); lives in the wv
                # pool so its teardown barrier lands after the v phase.
                zb_sb = wvp.tile([128, 2561], BF16)
                nc.sync.dma_start(out=zb_sb[:], in_=zbT[:])
                for k in range(2):
                    nc.sync.dma_start(out=bass.AP(tensor=zbP[k], offset=0,
                                                  ap=[[2561, 128], [1, 2561]]),
                                      in_=zb_sb[:])
                Wv1_sb = []
                for d in range(8):
                    t = wvp.tile([128, 2048], FP8, tag=f"wv1_{d}")
                    nc.gpsimd.dma_start(out=t[:], in_=WV1[128 * d:128 * (d + 1), :])
                    Wv1_sb.append(t)
                Wv2_sb = []
                for e in range(4):
                    t = wvp.tile([128, 2048], FP8, tag=f"wv2_{e}")
                    nc.gpsimd.dma_start(out=t[:], in_=WV2[128 * e:128 * (e + 1), :])
                    Wv2_sb.append(t)
                # Wqk on gpsimd, overlaps the v-projection compute
                Wqk1_sb = []
                for d in range(8):
                    t = pers.tile([128, 4096], FP8, name=f"wqk1{d}", tag=f"wqk1{d}")
                    nc.gpsimd.dma_start(out=t[:], in_=WQK1[128 * d:128 * (d + 1), :])
                    Wqk1_sb.append(t)
                Wqk2_sb = []
                for e in range(4):
                    t = pers.tile([128, 4096], FP8, name=f"wqk2{e}", tag=f"wqk2{e}")
                    nc.gpsimd.dma_start(out=t[:], in_=WQK2[128 * e:128 * (e + 1), :])
                    Wqk2_sb.append(t)
                for tt in range(8):
                    vt = vhat_sb[tt]
                    ones_ap = bass.AP(tensor=vt[:].tensor, offset=64,
                                      ap=[[16 * 65, 128], [65, 16]])
                    nc.vector.memset(ones_ap, 1.0)
                    for fc in range(2):
                        ps = mmp.tile([128, 512], F32, tag="mm")
                        for c2 in range(2):
                            fb = 512 * fc + 256 * c2
                            for d in range(8):
                                nc.tensor.matmul(
                                    ps[:, 256 * c2:256 * (c2 + 1)],
                                    x_stat(1024 * d, 8192, 128 * tt),
                                    bass.AP(tensor=Wv1_sb[d][:].tensor,
                                            offset=Wv1_sb[d][:].offset + fb,
                                            ap=[[2048, 128], [1024, 2], [1, 256]]),
                                    start=(d == 0), stop=False, perf_mode=DR,
                                )
                            for e in range(4):
                                nc.tensor.matmul(
                                    ps[:, 256 * c2:256 * (c2 + 1)],
                                    x_stat(2048 * e, 1024, 128 * tt),
                                    bass.AP(tensor=Wv2_sb[e][:].tensor,
                                            offset=Wv2_sb[e][:].offset + fb,
                                            ap=[[2048, 128], [1024, 2], [1, 256]]),
                                    start=False, stop=(e == 3), perf_mode=DR,
                                )
                        srcA = bass.AP(tensor=ps[:].tensor,
                                       offset=ps[:].offset,
                                       ap=[[512, 128], [64, 8], [1, 64]])
                        dst = bass.AP(tensor=vt[:].tensor, offset=65 * 8 * fc,
                                      ap=[[16 * 65, 128], [65, 8], [1, 64]])
                        if (tt + fc) % 2 == 0:
                            nc.scalar.activation(dst, srcA,
                                                 mybir.ActivationFunctionType.Copy,
                                                 scale=1.0 / 32.0)
                        else:
                            nc.vector.tensor_scalar_mul(dst, srcA, 1.0 / 32.0)

            pair_sb = [pers.tile([128, S], BF16, name=f"pair{hp}", tag=f"pair{hp}")
                       for hp in range(8)]
            Wp_sb = [pers.tile([128, D], BF16, name=f"wp{d}", tag=f"wp{d}")
                     for d in range(8)]

            # dt strips go through DRAM: the diagonal write covers the same
            # cells every pair; the zero background persists in DRAM.
            dtz = pers.tile([128, 160], BF16)
            nc.vector.memset(dtz[:], 0.0)
            for k2 in range(4):
                nc.gpsimd.dma_start(out=dtD[k2][:], in_=dtz[:])

            # broadcast out-proj bias across partitions once; the epilogue
            # then folds it into the eviction add instead of 16 PE matmuls
            for bc in range(2):
                psb_b = mmp.tile([128, 512], F32, tag="mm")
                nc.tensor.matmul(psb_b[:], ones_col[:],
                                 bp_sb[:, 512 * bc:512 * (bc + 1)],
                                 start=True, stop=True)
                nc.vector.tensor_copy(bpb_sb[:, 512 * bc:512 * (bc + 1)],
                                      psb_b[:])

            expT = pers.tile([128, 16 * 1024], BF16)
            # block-7 tail cols [128:144) per head are read by the ewP write but
            # never written by exps -> zero them once.
            nc.vector.memset(expT[:, 2048 * 7 + 128:2048 * 7 + 144], 0.0)
            nc.vector.memset(expT[:, 2048 * 7 + 1024 + 128:2048 * 7 + 1024 + 144], 0.0)

            # ---- attention ----
            with (
                tc.tile_pool(name="qk", bufs=2) as qkp,
                tc.tile_pool(name="dpp", bufs=2) as dpp,
                tc.tile_pool(name="bandp", bufs=2) as bandp,
                tc.tile_pool(name="eskp", bufs=2) as eskp,
                tc.tile_pool(name="denp", bufs=1) as denp,
                tc.tile_pool(name="pso", bufs=1, space="PSUM") as psop,
            ):
                def emit_qkproj_sec(hp, sec):
                    ftbase = 1024 * sec + 128 * hp
                    ft = 8 * sec + hp
                    dstt = qkp.tile([128, S], BF16, name=f"qk{sec}", tag=f"qk{sec}")
                    for tch in range(2):
                        ps = mmp.tile([128, 512], F32, tag="mm")
                        for c2 in range(2):
                            tb = 512 * tch + 256 * c2
                            for d in range(8):
                                nc.tensor.matmul(
                                    ps[:, 256 * c2:256 * (c2 + 1)],
                                    bass.AP(tensor=Wqk1_sb[d][:].tensor,
                                            offset=Wqk1_sb[d][:].offset + ftbase,
                                            ap=[[4096, 128], [2048, 2], [1, 128]]),
                                    x_mov(1024 * d, 8192, tb),
                                    start=(d == 0), stop=False, perf_mode=DR,
                                )
                            for e in range(4):
                                nc.tensor.matmul(
                                    ps[:, 256 * c2:256 * (c2 + 1)],
                                    bass.AP(tensor=Wqk2_sb[e][:].tensor,
                                            offset=Wqk2_sb[e][:].offset + ftbase,
                                            ap=[[4096, 128], [2048, 2], [1, 128]]),
                                    x_mov(2048 * e, 1024, tb),
                                    start=False, stop=(e == 3), perf_mode=DR,
                                )
                        if sec == 0:
                            with tc.high_priority(offset=1500):
                                nc.vector.tensor_scalar(
                                    dstt[:, 512 * tch:512 * (tch + 1)], ps[:],
                                    1.0 / 32.0, bqk_sb[:, ft:ft + 1],
                                    op0=mybir.AluOpType.mult,
                                    op1=mybir.AluOpType.add)
                        else:
                            # k bias is softmax-invariant (adds q_i.b_k, constant
                            # per row) -> plain evict copy, off the DVE
                            nc.vector.tensor_scalar_mul(
                                dstt[:, 512 * tch:512 * (tch + 1)], ps[:],
                                1.0 / 32.0)
                    return dstt

                def emit_band_part1(hp, qT_pair):
                    """dp strip matmuls + sheared dpSh DMA."""
                    dpT_h = []
                    for h in range(2):
                        dpT = dpp.tile([16, 1040], BF16, name=f"dpT{h}", tag=f"dpT{h}")
                        nc.vector.memset(dpT[:, 1024:1040], 0.0)
                        for tch in range(2):
                            psdp = mmp.tile([16, 512], F32, tag="mm")
                            nc.tensor.matmul(psdp[0:16, :],
                                             dlut_sb[64 * h:64 * h + 64, :],
                                             qT_pair[64 * h:64 * h + 64,
                                                     512 * tch:512 * (tch + 1)],
                                             start=True, stop=True)
                            nc.vector.tensor_copy(dpT[:, 512 * tch:512 * (tch + 1)],
                                                  psdp[0:16, :])
                        dpT_h.append(dpT)
                    dpSh = dpp.tile([32, 1024], BF16, tag="dpSh")
                    for h in range(2):
                        src = bass.AP(tensor=dpT_h[h][:].tensor, offset=0,
                                      ap=[[1041, 16], [1, 1024]])
                        nc.sync.dma_start(out=dpSh[16 * h:16 * h + 16, :], in_=src)
                    return dpSh

                def emit_band_part2(hp, dpSh):
                    """strip transposes + batched band write / masked read-back."""
                    dpS_all = dpp.tile([128, 256], BF16, tag="dpS")
                    for jt in range(8):
                        psd = mmp.tile([128, 32], BF16, tag="mm")
                        nc.tensor.matmul(psd[:, 0:32], dpSh[:, 128 * jt:128 * (jt + 1)],
                                         identb[0:32, 0:32], is_transpose=True,
                                         skip_group_check=True)
                        nc.vector.tensor_copy(dpS_all[:, 32 * jt:32 * (jt + 1)],
                                              psd[:, 0:32])
                    zb = zbP[hp % 2]
                    dstW = bass.AP(tensor=zb, offset=0,
                                   ap=[[2562, 128], [160, 16], [1, 16]])
                    srcW = bass.AP(tensor=dpS_all[:].tensor, offset=0,
                                   ap=[[256, 128], [16, 16], [1, 16]])
                    nc.gpsimd.dma_start(out=dstW, in_=srcW)
                    band = bandp.tile([128, 2560], BF16, tag="band")
                    for bh in range(2):
                        srcR = bass.AP(tensor=zb, offset=1280 * bh,
                                       ap=[[2561, 128], [1, 1280]])
                        nc.gpsimd.dma_start(out=band[:, 1280 * bh:1280 * (bh + 1)],
                                            in_=srcR)
                    return band

                def emit_scores(hp, h, qT_pair, kT_pair, band):
                    # primary (leading-512) blocks only: these cover the
                    # 144-wide band strips the ewP write needs, so the esk
                    # chain can launch before the tail exps run.
                    qT = qT_pair[64 * h:64 * h + 64, :]
                    kT = kT_pair[64 * h:64 * h + 64, :]
                    for jt in range(8):
                        j0 = 128 * jt
                        wd = min(512, S - j0)
                        win = min(144, S - j0)
                        ebase = 2048 * jt + 1024 * h
                        pss = mmp.tile([128, 512], F32, tag="mm")
                        nc.tensor.matmul(pss[:, 0:wd],
                                         kT[:, j0:j0 + 128],
                                         qT[:, j0:j0 + wd],
                                         start=True, stop=True)
                        with tc.high_priority(offset=1500):
                            nc.vector.tensor_add(
                                pss[:, 0:win], pss[:, 0:win],
                                band[:, 320 * jt + 160 * h:320 * jt + 160 * h + win])
                        with tc.high_priority(offset=1500):
                            nc.scalar.activation(expT[:, ebase:ebase + wd],
                                                 pss[:, 0:wd], EXPF)

                def emit_scores_tail(hp, h, qT_pair, kT_pair):
                    qT = qT_pair[64 * h:64 * h + 64, :]
                    kT = kT_pair[64 * h:64 * h + 64, :]
                    for jt in range(4):
                        j0 = 128 * jt
                        ebase = 2048 * jt + 1024 * h
                        w1 = S - j0 - 512
                        pss1 = mmp.tile([128, 512], F32, tag="mm")
                        nc.tensor.matmul(pss1[:, 0:w1],
                                         kT[:, j0:j0 + 128],
                                         qT[:, j0 + 512:S],
                                         start=True, stop=True)
                        nc.scalar.activation(expT[:, ebase + 512:ebase + 512 + w1],
                                             pss1[:, 0:w1], EXPF)

                def emit_attnv(hp, h, pso):
                    hg = 2 * hp + h
                    for jt in range(8):
                        j0 = 128 * jt
                        ebase = 2048 * jt + 1024 * h
                        lhs = vhat_sb[jt][:, 65 * hg:65 * hg + 65]
                        segs = ([(j0, 512), (512, 1024)] if j0 < 512
                                else [(j0, 1024)])
                        for (a, b2) in segs:
                            nc.tensor.matmul(pso[:, a:b2], lhs,
                                             expT[:, ebase + a - j0:ebase + b2 - j0],
                                             start=(jt == 0), stop=False,
                                             skip_group_check=True)

                # prologue: pair 0 qk + band chain
                qk_q0 = emit_qkproj_sec(0, 0)
                dpSh0 = emit_band_part1(0, qk_q0)
                qk_next = [qk_q0, emit_qkproj_sec(0, 1)]
                band_next = emit_band_part2(0, dpSh0)
                qk_q = emit_qkproj_sec(1, 0)
                dpSh_next = emit_band_part1(1, qk_q)

                dt_tiles = [None, None]
                recip_h = [None, None]

                def emit_recip(h):
                    rec = denp.tile([1, 1024], F32R, name=f"rec{h}", tag=f"rec{h}")
                    with nc.allow_low_precision(reason="denom recip broadcast"):
                        nc.vector.reciprocal(rec[:], pso_h[h][64:65, :])
                    recip_h[h] = rec

                def emit_esk(hp, h):
                    ew = ewP[2 * (hp % 2) + h]
                    dstE = bass.AP(tensor=ew, offset=0,
                                   ap=[[1152, 128], [144, 8], [1, 144]])
                    srcE = bass.AP(tensor=expT[:].tensor, offset=1024 * h,
                                   ap=[[16384, 128], [2048, 8], [1, 144]])
                    nc.sync.dma_start(out=dstE, in_=srcE)
                    esk = eskp.tile([128, 128], BF16, name=f"esk{h}", tag=f"esk{h}")
                    srcK = bass.AP(tensor=ew, offset=0,
                                   ap=[[1153, 128], [144, 8], [1, 16]])
                    nc.sync.dma_start(out=esk[:], in_=srcK)
                    return esk

                def emit_strip_chain(hp, h, esk):
                    psT = mmp.tile([128, 128], BF16, tag="mm")
                    nc.tensor.matmul(psT[:, :], esk[:], identb[:, :],
                                     is_transpose=True, skip_group_check=True)
                    eskT = eskp.tile([128, 128], BF16, name=f"eskT{h}",
                                     tag=f"eskT{h}")
                    nc.vector.tensor_copy(eskT[:], psT[:, :])
                    dtd = dtD[2 * (hp % 2) + h]
                    eng = nc.sync
                    eng.dma_start(out=bass.AP(tensor=dtd, offset=0,
                                              ap=[[160, 128], [1, 128]]),
                                  in_=eskT[:])
                    dt_t = eskp.tile([16, 1280], BF16, name=f"dt{h}", tag=f"dt{h}")
                    srcR = bass.AP(tensor=dtd, offset=0,
                                   ap=[[159, 16], [2560, 8], [1, 160]])
                    dstR = bass.AP(tensor=dt_t[:].tensor, offset=0,
                                   ap=[[1280, 16], [160, 8], [1, 160]])
                    eng.dma_start(out=dstR, in_=srcR)
                    dt_tiles[h] = dt_t

                def emit_psb(h):
                    psb_cs = []
                    for c in range(2):
                        psb = mmp.tile([64, 512], F32, tag="mm")
                        nc.tensor.matmul(psb[0:64, :], ones_r[:],
                                         recip_h[h][:, 512 * c:512 * (c + 1)],
                                         start=True, stop=True)
                        psb_cs.append(psb)
                    return psb_cs

                def emit_dlv(h):
                    pso = pso_h[h]
                    for jt in range(8):
                        j0 = 128 * jt
                        win = min(144, S - j0)
                        a0, b0 = j0, j0 + win
                        dsegs = ([(a0, 512), (512, b0)] if (a0 < 512 < b0)
                                 else [(a0, b0)])
                        for (a, b2) in dsegs:
                            nc.tensor.matmul(
                                pso[0:64, a:b2], dlv_sb[:],
                                dt_tiles[h][:, 160 * jt + a - j0:160 * jt + b2 - j0],
                                start=False,
                                stop=(jt == 7 and (a, b2) == dsegs[-1]),
                                skip_group_check=True)

                def emit_evict_mul(hp, h, psb_cs):
                    for c in range(2):
                        nc.scalar.copy(
                            pair_sb[hp][64 * h:64 * h + 64, 512 * c:512 * (c + 1)],
                            pso_h[h][0:64, 512 * c:512 * (c + 1)])
                        nc.vector.tensor_mul(
                            pair_sb[hp][64 * h:64 * h + 64, 512 * c:512 * (c + 1)],
                            pair_sb[hp][64 * h:64 * h + 64, 512 * c:512 * (c + 1)],
                            psb_cs[c][0:64, :])

                # prologue covers pair 0; loop prefetches pair p+1's k-proj /
                # pair p+2's q-proj inside pair p's tail as stall filler.
                # Scores for both heads run first (exps on Act overlap), then
                # the attnv bursts; the strip chains fly over the qkproj/band
                # filler so dlv/evict land without stalling PE.
                qk_k = qk_next[1]
                for hp in range(8):
                    qT_pair, kT_pair = qk_next
                    band = band_next
                    pso_h = [psop.tile([65, 1024], F32, name=f"pso{h}", tag=f"pso{h}")
                             for h in range(2)]
                    emit_scores(hp, 0, qT_pair, kT_pair, band)
                    with tc.high_priority(offset=6000):
                        esk0 = emit_esk(hp, 0)
                    emit_scores(hp, 1, qT_pair, kT_pair, band)
                    with tc.high_priority(offset=6000):
                        esk1 = emit_esk(hp, 1)
                    emit_scores_tail(hp, 0, qT_pair, kT_pair)
                    emit_scores_tail(hp, 1, qT_pair, kT_pair)
                    emit_attnv(hp, 0, pso_h[0])
                    emit_attnv(hp, 1, pso_h[1])
                    with tc.high_priority(offset=6000):
                        emit_strip_chain(hp, 0, esk0)
                        emit_recip(0)
                        emit_recip(1)
                    if hp < 7:
                        with tc.high_priority(offset=6000):
                            band_next = emit_band_part2(hp + 1, dpSh_next)
                        qk_k = emit_qkproj_sec(hp + 1, 1)
                    with tc.high_priority(offset=6000):
                        psb0 = emit_psb(0)
                        emit_dlv(0)
                        emit_evict_mul(hp, 0, psb0)
                        emit_strip_chain(hp, 1, esk1)
                    if hp < 7:
                        qk_next = [qk_q, qk_k]
                        if hp < 6:
                            qk_q = emit_qkproj_sec(hp + 2, 0)
                            dpSh_next = emit_band_part1(hp + 2, qk_q)
                    with tc.high_priority(offset=6000):
                        psb1 = emit_psb(1)
                        emit_dlv(1)
                        emit_evict_mul(hp, 1, psb1)
                    if hp == 3:
                        for d in range(8):
                            nc.gpsimd.dma_start(out=Wp_sb[d][:],
                                                in_=Wp[128 * d:128 * (d + 1), :])

            # ---- final projection ----
            with (
                tc.tile_pool(name="ps_p", bufs=2, space="PSUM") as ps_p,
                tc.tile_pool(name="outp", bufs=2) as outp,
            ):
                for tt in range(8):
                    ps = ps_p.tile([128, 1024], F32, tag="psp")
                    for fc in range(2):
                        for d in range(8):
                            nc.tensor.matmul(
                                ps[:, 512 * fc:512 * (fc + 1)],
                                pair_sb[d][:, 128 * tt:128 * (tt + 1)],
                                Wp_sb[d][:, 512 * fc:512 * (fc + 1)],
                                start=(d == 0), stop=(d == 7),
                            )
                    ot = outp.tile([128, 1024], BF16, tag="ot")
                    nc.vector.tensor_add(ot[:], ps[:], bpb_sb[:])
                    nc.sync.dma_start(out=OUT[128 * tt:128 * (tt + 1), :], in_=ot[:])

    nc.compile()
    return nc


def _host_prep(W_attn, b_attn, W_proj, b_proj, lut_k, lut_v):
    scale = 1.0 / math.sqrt(d_k)
    Wqk_h = np.concatenate([W_attn[:, :D], W_attn[:, D:2 * D] * scale], axis=1)
    bq = b_attn[:D]
    bk = b_attn[D:2 * D] * scale
    bqk_h = np.stack([np.concatenate([bq, bk])[128 * ft:128 * (ft + 1)]
                      for ft in range(16)], axis=1).astype(np.float32)
    bv = b_attn[2 * D:3 * D] + np.tile(lut_v[0], N_H)
    bp_h = (np.asarray(b_proj) + bv @ W_proj).reshape(1, D)
    dlut_h = np.stack([(lut_k[16 - u] - lut_k[0]) * scale for u in range(16)],
                      axis=1)
    dlv_h = np.stack([lut_v[16 - u] - lut_v[0] for u in range(16)], axis=0)
    # zbP template: flat[2561*p + col], col = 160*jh + c (16 blocks of 160):
    # c < p -> MASKVAL (causal), c >= p+16 -> 0; band cells get overwritten.
    c_of_col = (np.arange(2561) % 160)[None, :]
    pvec = np.arange(128)[:, None]
    zbT_h = np.where(c_of_col < pvec, np.float32(MASKVAL), np.float32(0.0))
    # fp8 3-term residual split: xW ~= x1W1 + x2W1 + x1W2 (x2W2 dropped).
    # W is scaled by 32 into e4m3's normal range; PSUM evicts divide by 32.
    Wqk_s = Wqk_h * 32.0
    Wqk1 = Wqk_s.astype(E4)
    Wqk2 = (Wqk_s - Wqk1.astype(np.float32)).astype(E4)
    # P1 stationary: per d-chunk, the W1 block duplicated (pair-major [2, 2048])
    wqk1_t = np.concatenate(
        [Wqk1.reshape(8, 128, 2048)[:, :, None, :]] * 2, axis=2
    ).reshape(D, 4096)
    # P2 stationary: pairs (W2_{2e}, W2_{2e+1})
    wqk2_t = Wqk2.reshape(4, 2, 128, 2048).transpose(0, 2, 1, 3).reshape(512, 4096)
    Wv_s = np.ascontiguousarray(W_attn[:, 2 * D:3 * D]) * 32.0
    Wv1 = Wv_s.astype(E4)
    Wv2 = (Wv_s - Wv1.astype(np.float32)).astype(E4)
    wv1_t = np.concatenate(
        [Wv1.reshape(8, 128, 1024)[:, :, None, :]] * 2, axis=2
    ).reshape(D, 2048)
    wv2_t = Wv2.reshape(4, 2, 128, 1024).transpose(0, 2, 1, 3).reshape(512, 2048)
    return {
        "WQK1": wqk1_t.view(np.uint8),
        "WQK2": wqk2_t.view(np.uint8),
        "WV1": wv1_t.view(np.uint8),
        "WV2": wv2_t.view(np.uint8),
        "Wp": np.ascontiguousarray(W_proj).astype(BF),
        "bqk": bqk_h,
        "bprow": bp_h.astype(BF),
        "dlut": dlut_h.astype(BF),
        "dlv": dlv_h.astype(BF),
        "zbT": zbT_h.astype(BF),
    }


def kernel(x, W_attn, b_attn, W_proj, b_proj, lut_k, lut_v):
    x = np.asarray(x, np.float32)
    shared = _host_prep(np.asarray(W_attn, np.float32),
                        np.asarray(b_attn, np.float32),
                        np.asarray(W_proj, np.float32),
                        np.asarray(b_proj, np.float32),
                        np.asarray(lut_k, np.float32),
                        np.asarray(lut_v, np.float32))
    if "nc" not in _CACHE:
        _CACHE["nc"] = build_module()
    nc = _CACHE["nc"]
    in_maps = []
    for b in range(N_CORES):
        m = dict(shared)
        xT_f = np.ascontiguousarray(x[b].T)  # [1024 d, 1024 t] f32
        x1 = xT_f.astype(E4)
        x2 = (xT_f - x1.astype(np.float32)).astype(E4)
        # [128 p, 8 d-chunks, 1024 t] -> [128, 8192] per level, concat levels
        p1 = x1.reshape(8, 128, 1024).transpose(1, 0, 2).reshape(128, 8192)
        p2 = x2.reshape(8, 128, 1024).transpose(1, 0, 2).reshape(128, 8192)
        m["X8"] = np.concatenate([p1, p2], axis=1).view(np.uint8)
        in_maps.append(m)
    res = run_bass_kernel_spmd(nc, in_maps, list(range(N_CORES)), trace=TRACE)
    _CACHE["last_result"] = res
    out = np.stack([res.results[b]["OUT"] for b in range(N_CORES)], axis=0)
    return out.astype(np.float32)

